# revision 5
# baseline (speedup 1.0000x reference)
"""Multi-head attention + output Linear on 8 Trainium2 NeuronCores.

Problem: bs=2, seq=2048, embed=1024, heads=16, head_dim=64.
  out = Linear(softmax(mask(Q K^T / 8)) V)        (eval-mode dropout)

Sharding: core c in 0..7 handles batch b = c//4 and query block qb = c%4
(512 query rows), computing its exact [512, 1024] output slice — heads stay
together per core so the output Linear needs no cross-core reduction.

Per-core kernel (Tile framework), all matmuls in float32r (1 cyc/row):
  scoresT[k, q] = K_h Q_h^T   (contraction over d=64, partition dim)
  probsT = exp(scoresT / 8) * maskT        (ACT exp fused scale; DVE/GPSIMD mul)
  outT[65, q]  = [V_h | 1]^T probsT        (ones column yields softmax denom)
  attnT = outT[0:64] * (1 / outT[64])      (normalize on evict)
  y = attnT^T W^T + bias                   (accumulate all 16 heads)
"""

import sys
import numpy as np

sys.path.insert(0, "/opt/trn_rl_repo")

import concourse.bass as bass
import concourse.tile as tile
from concourse import bacc, mybir
from concourse.bass_utils import run_bass_kernel_spmd

BS, SEQ, EMBED, HEADS = 2, 2048, 1024, 16
D = EMBED // HEADS            # 64
QB = SEQ // 4                 # 512 query rows per core
NC_COUNT = 8
KC = SEQ // 128               # 16 k chunks
F32 = mybir.dt.float32
F32R = mybir.dt.float32r

_CACHE = {}


def _build_nc():
    nc = bacc.Bacc("TRN2", target_bir_lowering=False, debug=False)

    qT = nc.dram_tensor("qT", [HEADS, D, QB], F32R, kind="ExternalInput")
    kT = nc.dram_tensor("kT", [HEADS, D, SEQ], F32R, kind="ExternalInput")
    vb = nc.dram_tensor("vb", [SEQ, EMBED], F32R, kind="ExternalInput")
    mT = nc.dram_tensor("mT", [SEQ, QB], F32, kind="ExternalInput")
    WT = nc.dram_tensor("WT", [EMBED, EMBED], F32R, kind="ExternalInput")
    bias = nc.dram_tensor("bias", [EMBED], F32, kind="ExternalInput")
    ones = nc.dram_tensor("ones", [128, KC], F32R, kind="ExternalInput")
    y = nc.dram_tensor("y", [QB, EMBED], F32, kind="ExternalOutput")

    with tile.TileContext(nc) as tc:
        with tc.tile_pool(name="const", bufs=1) as const, \
             tc.tile_pool(name="kpool", bufs=2) as kpool, \
             tc.tile_pool(name="vpool", bufs=2) as vpool, \
             tc.tile_pool(name="probs", bufs=4) as probs, \
             tc.tile_pool(name="small", bufs=4) as small, \
             tc.tile_pool(name="ypool", bufs=2) as ypool, \
             tc.tile_pool(name="dscr", bufs=2, space="DRAM") as dscr, \
             tc.tile_pool(name="scps", bufs=2, space="PSUM") as scps, \
             tc.tile_pool(name="accps", bufs=2, space="PSUM") as accps:

            # ---- constants ----
            WT_sb = const.tile([128, 8, EMBED], F32R)
            nc.sync.dma_start(out=WT_sb, in_=WT.rearrange("(c p) e -> p c e", p=128))
            mT_sb = const.tile([128, KC, QB], F32)
            nc.sync.dma_start(out=mT_sb, in_=mT.rearrange("(c p) q -> p c q", p=128))
            qT_sb = const.tile([D, HEADS, QB], F32R)
            nc.sync.dma_start(out=qT_sb, in_=qT.rearrange("h d q -> d h q"))
            bias_ap = bias[:]
            bias_bc = const.tile([128, EMBED], F32)
            nc.gpsimd.dma_start(
                out=bias_bc,
                in_=bass.AP(tensor=bias_ap.tensor, offset=bias_ap.offset,
                            ap=[[0, 128]] + list(bias_ap.ap)),
            )
            attnT = const.tile([128, 8, QB], F32R)

            v_re = vb.rearrange("(c p) e -> p c e", p=128)

            for h in range(HEADS):
                kT_t = kpool.tile([D, SEQ], F32R)
                nc.sync.dma_start(out=kT_t, in_=kT[h])
                va = vpool.tile([128, KC, D + 1], F32R)
                nc.sync.dma_start(out=va[:, :, 0:D], in_=v_re[:, :, h * D:(h + 1) * D])
                nc.sync.dma_start(out=va[:, :, D], in_=ones[:, :])

                outT = accps.tile([D + 1, QB], F32)
                for g in range(8):           # chunk pairs
                    sc = scps.tile([128, 2, QB], F32)
                    for j in range(2):
                        c = 2 * g + j
                        nc.tensor.matmul(sc[:, j, :],
                                         kT_t[:, c * 128:(c + 1) * 128],
                                         qT_sb[:, h, :],
                                         start=True, stop=True)
                    pe_t = probs.tile([128, 2, QB], F32R)
                    nc.scalar.activation(out=pe_t, in_=sc,
                                         func=mybir.ActivationFunctionType.Exp,
                                         scale=float(1.0 / np.sqrt(D)))
                    eng = nc.gpsimd if g % 3 == 0 else nc.vector
                    eng.tensor_mul(pe_t, pe_t, mT_sb[:, 2 * g:2 * g + 2, :])
                    for j in range(2):
                        c = 2 * g + j
                        nc.tensor.matmul(outT, va[:, c, :], pe_t[:, j, :],
                                         start=(c == 0), stop=(c == KC - 1))

                rc = small.tile([1, QB], F32)
                nc.vector.reciprocal(rc, outT[D:D + 1, :])
                # SBUF APs can't broadcast along partitions; bounce via DRAM.
                dr = dscr.tile([1, QB], F32)
                nc.gpsimd.dma_start(out=dr, in_=rc)
                rb = small.tile([D, QB], F32)
                dr_ap = dr[0:1, :]
                nc.gpsimd.dma_start(
                    out=rb,
                    in_=bass.AP(tensor=dr_ap.tensor, offset=dr_ap.offset,
                                ap=[[0, D]] + list(dr_ap.ap)[1:]),
                )
                nc.vector.tensor_mul(
                    attnT[(h % 2) * D:(h % 2) * D + D, h // 2, :],
                    outT[0:D, :], rb)

            # ---- output linear ----
            for qc in range(4):
                y_sb = ypool.tile([128, EMBED], F32)
                for n in range(2):
                    ps = accps.tile([128, 512], F32)
                    for kc in range(8):
                        nc.tensor.matmul(ps,
                                         attnT[:, kc, qc * 128:(qc + 1) * 128],
                                         WT_sb[:, kc, n * 512:(n + 1) * 512],
                                         start=(kc == 0), stop=(kc == 7))
                    nc.vector.tensor_add(y_sb[:, n * 512:(n + 1) * 512], ps,
                                         bias_bc[:, n * 512:(n + 1) * 512])
                nc.sync.dma_start(out=y[qc * 128:(qc + 1) * 128, :], in_=y_sb)

    nc.compile()
    return nc


def _prep_in_maps(q, k, v, padding_mask, W, b):
    q = np.asarray(q, dtype=np.float32)
    k = np.asarray(k, dtype=np.float32)
    v = np.asarray(v, dtype=np.float32)
    m = np.asarray(padding_mask)
    W = np.asarray(W, dtype=np.float32)
    b = np.asarray(b, dtype=np.float32)

    # [bs, seq, embed] -> [bs, heads, d, seq]
    qT = np.ascontiguousarray(q.reshape(BS, SEQ, HEADS, D).transpose(0, 2, 3, 1))
    kT = np.ascontiguousarray(k.reshape(BS, SEQ, HEADS, D).transpose(0, 2, 3, 1))
    # mask [bs, 1, q, k] -> float [bs, k, q]
    mT = np.ascontiguousarray(m[:, 0].transpose(0, 2, 1).astype(np.float32))
    WTc = np.ascontiguousarray(W.T)

    in_maps = []
    for c in range(NC_COUNT):
        bi, qb = c // 4, c % 4
        in_maps.append({
            "qT": np.ascontiguousarray(qT[bi, :, :, qb * QB:(qb + 1) * QB]),
            "kT": kT[bi],
            "vb": v[bi],
            "mT": np.ascontiguousarray(mT[bi, :, qb * QB:(qb + 1) * QB]),
            "WT": WTc,
            "bias": b,
            "ones": np.ones((128, KC), dtype=np.float32),
        })
    return in_maps


def _run(in_maps, **kw):
    if "nc" not in _CACHE:
        _CACHE["nc"] = _build_nc()
    return run_bass_kernel_spmd(_CACHE["nc"], in_maps, list(range(NC_COUNT)), **kw)


def kernel(q, k, v, padding_mask, W, b):
    in_maps = _prep_in_maps(q, k, v, padding_mask, W, b)
    res = _run(in_maps)
    out = np.empty((BS, SEQ, EMBED), dtype=np.float32)
    for c in range(NC_COUNT):
        bi, qb = c // 4, c % 4
        out[bi, qb * QB:(qb + 1) * QB] = res.results[c]["y"]
    return out


def kernel_traced(q, k, v, padding_mask, W, b):
    """Like kernel() but returns (out, BassKernelResults-with-trace)."""
    in_maps = _prep_in_maps(q, k, v, padding_mask, W, b)
    res = _run(in_maps, trace=True)
    out = np.empty((BS, SEQ, EMBED), dtype=np.float32)
    for c in range(NC_COUNT):
        bi, qb = c // 4, c % 4
        out[bi, qb * QB:(qb + 1) * QB] = res.results[c]["y"]
    return out, res


# revision 12
# speedup vs baseline: 1.0669x; 1.0669x over previous
"""Multi-head attention + output Linear on 8 Trainium2 NeuronCores.

Problem: bs=2, seq=2048, embed=1024, heads=16, head_dim=64.
  out = Linear(softmax(mask(Q K^T / 8)) V)        (eval-mode dropout)

Sharding: core c in 0..7 handles batch b = c//4 and query block qb = c%4
(512 query rows), computing its exact [512, 1024] output slice - heads stay
together per core so the output Linear needs no cross-core reduction.

Per-core kernel (Tile framework), all matmuls in float32r (1 cyc/row):
  scoresT[k, q] = K_h Q_h^T   (contraction over d=64, partition dim)
  probsT = exp(scoresT / 8) * maskT        (ACT exp fused scale; DVE/GPSIMD mul)
  outT[65, q]  = [V_h | 1]^T probsT        (ones column yields softmax denom)
  attnT = outT[0:64] * (1 / outT[64])      (denom broadcast via PE outer product)
  y = attnT^T W^T + bias                   (accumulate all 16 heads)

Heads are processed in pairs so K / V DMAs move 512B+ descriptors; the
[V | 1] lhsT tiles are assembled on-chip by DVE copies (tiny strided DMAs
were the dominant modeled cost).
"""

import sys
import numpy as np

sys.path.insert(0, "/opt/trn_rl_repo")

import concourse.bass as bass
import concourse.tile as tile
from concourse import bacc, mybir
from concourse.bass_utils import run_bass_kernel_spmd

BS, SEQ, EMBED, HEADS = 2, 2048, 1024, 16
D = EMBED // HEADS            # 64
QB = SEQ // 4                 # 512 query rows per core
NC_COUNT = 8
KC = SEQ // 128               # 16 k chunks
F32 = mybir.dt.float32
F32R = mybir.dt.float32r

_CACHE = {}


def _build_nc(scps_bufs=3, accps_bufs=2, probs_bufs=6, gps_mod=3, kpool_bufs=2,
              vpool_bufs=2, vapool_bufs=2, small_bufs=4, ypool_bufs=2,
              exp_group=2):
    nc = bacc.Bacc("TRN2", target_bir_lowering=False, debug=False)

    qT = nc.dram_tensor("qT", [HEADS, D, QB], F32R, kind="ExternalInput")
    kT = nc.dram_tensor("kT", [HEADS, D, SEQ], F32R, kind="ExternalInput")
    vb = nc.dram_tensor("vb", [SEQ, EMBED], F32R, kind="ExternalInput")
    mT = nc.dram_tensor("mT", [SEQ, QB], F32, kind="ExternalInput")
    WT = nc.dram_tensor("WT", [EMBED, EMBED], F32R, kind="ExternalInput")
    bias = nc.dram_tensor("bias", [EMBED], F32, kind="ExternalInput")
    ones = nc.dram_tensor("ones", [128, D], F32R, kind="ExternalInput")
    y = nc.dram_tensor("y", [QB, EMBED], F32, kind="ExternalOutput")

    ngrp = KC // exp_group    # exp groups per head

    with tile.TileContext(nc) as tc, \
         nc.allow_low_precision(reason="float32r matmul inputs; fp32 accumulate in PSUM"):
        with tc.tile_pool(name="const", bufs=1) as const, \
             tc.tile_pool(name="kpool", bufs=kpool_bufs) as kpool, \
             tc.tile_pool(name="vpool", bufs=vpool_bufs) as vpool, \
             tc.tile_pool(name="vapool", bufs=vapool_bufs) as vapool, \
             tc.tile_pool(name="probs", bufs=probs_bufs) as probs, \
             tc.tile_pool(name="small", bufs=small_bufs) as small, \
             tc.tile_pool(name="ypool", bufs=ypool_bufs) as ypool, \
             tc.tile_pool(name="scps", bufs=scps_bufs, space="PSUM") as scps, \
             tc.tile_pool(name="accps", bufs=accps_bufs, space="PSUM") as accps:

            # ---- constants ----
            WT_sb = const.tile([128, 8, EMBED], F32R)
            nc.sync.dma_start(out=WT_sb, in_=WT.rearrange("(c p) e -> p c e", p=128))
            mT_sb = const.tile([128, KC, QB], F32)
            nc.sync.dma_start(out=mT_sb, in_=mT.rearrange("(c p) q -> p c q", p=128))
            qT_sb = const.tile([128, 8, QB], F32R)
            nc.sync.dma_start(
                out=qT_sb,
                in_=qT.rearrange("(hp two) d q -> (two d) hp q", two=2))
            bias_ap = bias[:]
            bias_bc = const.tile([128, EMBED], F32)
            nc.sync.dma_start(
                out=bias_bc,
                in_=bass.AP(tensor=bias_ap.tensor, offset=bias_ap.offset,
                            ap=[[0, 128]] + list(bias_ap.ap)),
            )
            ones_sb = const.tile([128, D], F32R)
            nc.sync.dma_start(out=ones_sb, in_=ones[:, :])
            attnT = const.tile([128, 8, QB], F32R)

            v_re = vb.rearrange("(c p) e -> p c e", p=128)

            for hp in range(8):            # head pairs
                kTp = kpool.tile([128, SEQ], F32R)
                nc.sync.dma_start(
                    out=kTp,
                    in_=kT[2 * hp:2 * hp + 2].rearrange("h d s -> (h d) s"))
                vp = vpool.tile([128, KC, 128], F32R)
                nc.sync.dma_start(out=vp,
                                  in_=v_re[:, :, hp * 128:(hp + 1) * 128])
                # assemble [V_h | 1] lhsT tiles on-chip (cheap DVE copies)
                va = vapool.tile([128, KC, 2, D + 1], F32R)
                nc.vector.tensor_copy(va[:, :, 0, 0:D], vp[:, :, 0:D])
                nc.vector.tensor_copy(va[:, :, 1, 0:D], vp[:, :, D:2 * D])
                nc.vector.tensor_copy(va[:, :, :, D], ones_sb[:, 0:KC * 2])

                for hh in range(2):
                    h = 2 * hp + hh
                    outT = accps.tile([D + 1, QB], F32, tag="acc")
                    for g in range(ngrp):
                        sc = scps.tile([128, exp_group, QB], F32, tag="sc")
                        for j in range(exp_group):
                            c = exp_group * g + j
                            nc.tensor.matmul(
                                sc[:, j, :],
                                kTp[hh * D:hh * D + D, c * 128:(c + 1) * 128],
                                qT_sb[hh * D:hh * D + D, hp, :],
                                start=True, stop=True)
                        pe_t = probs.tile([128, exp_group, QB], F32R, tag="pe")
                        nc.scalar.activation(out=pe_t, in_=sc,
                                             func=mybir.ActivationFunctionType.Exp,
                                             scale=float(1.0 / np.sqrt(D)))
                        eng = nc.gpsimd if (gps_mod and g % gps_mod == 0) else nc.vector
                        eng.tensor_mul(
                            pe_t, pe_t,
                            mT_sb[:, exp_group * g:exp_group * (g + 1), :])
                        for j in range(exp_group):
                            c = exp_group * g + j
                            nc.tensor.matmul(outT, va[:, c, hh, :], pe_t[:, j, :],
                                             start=(c == 0), stop=(c == KC - 1))

                    # normalize: recip of denom row, broadcast via PE outer
                    # product (ones64 x recip), evict+scale on DVE
                    rc = small.tile([1, QB], F32R, tag="rc")
                    nc.vector.reciprocal(rc, outT[D:D + 1, :])
                    rb_ps = accps.tile([D, QB], F32, tag="acc")
                    nc.tensor.matmul(rb_ps, ones_sb[0:1, 0:D], rc[0:1, :],
                                     start=True, stop=True)
                    rb_sb = small.tile([D, QB], F32, tag="rb")
                    nc.vector.tensor_copy(rb_sb, rb_ps)
                    nc.vector.tensor_mul(
                        attnT[hh * D:hh * D + D, hp, :],
                        outT[0:D, :], rb_sb)

            # ---- output linear ----
            for qc in range(4):
                y_sb = ypool.tile([128, EMBED], F32)
                for n in range(2):
                    ps = accps.tile([128, 512], F32, tag="acc")
                    for kc in range(8):
                        nc.tensor.matmul(ps,
                                         attnT[:, kc, qc * 128:(qc + 1) * 128],
                                         WT_sb[:, kc, n * 512:(n + 1) * 512],
                                         start=(kc == 0), stop=(kc == 7))
                    nc.vector.tensor_add(y_sb[:, n * 512:(n + 1) * 512], ps,
                                         bias_bc[:, n * 512:(n + 1) * 512])
                nc.sync.dma_start(out=y[qc * 128:(qc + 1) * 128, :], in_=y_sb)

    nc.compile()
    return nc


def _prep_in_maps(q, k, v, padding_mask, W, b):
    q = np.asarray(q, dtype=np.float32)
    k = np.asarray(k, dtype=np.float32)
    v = np.asarray(v, dtype=np.float32)
    m = np.asarray(padding_mask)
    W = np.asarray(W, dtype=np.float32)
    b = np.asarray(b, dtype=np.float32)

    # [bs, seq, embed] -> [bs, heads, d, seq]
    qT = np.ascontiguousarray(q.reshape(BS, SEQ, HEADS, D).transpose(0, 2, 3, 1))
    kT = np.ascontiguousarray(k.reshape(BS, SEQ, HEADS, D).transpose(0, 2, 3, 1))
    # mask [bs, 1, q, k] -> float [bs, k, q]
    mT = np.ascontiguousarray(m[:, 0].transpose(0, 2, 1).astype(np.float32))
    WTc = np.ascontiguousarray(W.T)

    in_maps = []
    for c in range(NC_COUNT):
        bi, qb = c // 4, c % 4
        in_maps.append({
            "qT": np.ascontiguousarray(qT[bi, :, :, qb * QB:(qb + 1) * QB]),
            "kT": kT[bi],
            "vb": v[bi],
            "mT": np.ascontiguousarray(mT[bi, :, qb * QB:(qb + 1) * QB]),
            "WT": WTc,
            "bias": b,
            "ones": np.ones((128, D), dtype=np.float32),
        })
    return in_maps


def _run(in_maps, **kw):
    if "nc" not in _CACHE:
        _CACHE["nc"] = _build_nc()
    return run_bass_kernel_spmd(_CACHE["nc"], in_maps, list(range(NC_COUNT)), **kw)


def kernel(q, k, v, padding_mask, W, b):
    in_maps = _prep_in_maps(q, k, v, padding_mask, W, b)
    res = _run(in_maps)
    out = np.empty((BS, SEQ, EMBED), dtype=np.float32)
    for c in range(NC_COUNT):
        bi, qb = c // 4, c % 4
        out[bi, qb * QB:(qb + 1) * QB] = res.results[c]["y"]
    return out


# revision 13
# speedup vs baseline: 16698.9209x; 15651.6840x over previous
"""Multi-head attention + output Linear on 8 Trainium2 NeuronCores.

Problem: bs=2, seq=2048, embed=1024, heads=16, head_dim=64.
  out = Linear(softmax(mask(Q K^T / 8)) V)        (eval-mode dropout)

Sharding: core c in 0..7 handles batch b = c//4 and query block qb = c%4
(512 query rows), computing its exact [512, 1024] output slice - heads stay
together per core so the output Linear needs no cross-core reduction.

Per-core kernel (Tile framework), all matmuls in float32r (1 cyc/row):
  scoresT[k, q] = K_h Q_h^T   (contraction over d=64, partition dim)
  probsT = exp(scoresT / 8) * maskT        (ACT exp fused scale; DVE/GPSIMD mul)
  outT[65, q]  = [V_h | 1]^T probsT        (ones column yields softmax denom)
  attnT = outT[0:64] * (1 / outT[64])      (denom broadcast via PE outer product)
  y = attnT^T W^T + bias                   (accumulate all 16 heads)

Heads are processed in pairs so K / V DMAs move 512B+ descriptors; the
[V | 1] lhsT tiles are assembled on-chip by DVE copies (tiny strided DMAs
were the dominant modeled cost).
"""

import sys
import numpy as np

sys.path.insert(0, "/opt/trn_rl_repo")

import concourse.bass as bass
import concourse.tile as tile
from concourse import bacc, mybir
from concourse.bass_utils import run_bass_kernel_spmd

BS, SEQ, EMBED, HEADS = 2, 2048, 1024, 16
D = EMBED // HEADS            # 64
QB = SEQ // 4                 # 512 query rows per core
NC_COUNT = 8
KC = SEQ // 128               # 16 k chunks
F32 = mybir.dt.float32
F32R = mybir.dt.float32r

_CACHE = {}


def _build_nc(scps_bufs=3, accps_bufs=2, probs_bufs=6, gps_mod=3, kpool_bufs=2,
              vpool_bufs=2, vapool_bufs=2, small_bufs=4, ypool_bufs=2,
              exp_group=2, interleave=False, il_probs_bufs=3):
    nc = bacc.Bacc("TRN2", target_bir_lowering=False, debug=False)

    qT = nc.dram_tensor("qT", [HEADS, D, QB], F32R, kind="ExternalInput")
    kT = nc.dram_tensor("kT", [HEADS, D, SEQ], F32R, kind="ExternalInput")
    vb = nc.dram_tensor("vb", [SEQ, EMBED], F32R, kind="ExternalInput")
    mT = nc.dram_tensor("mT", [SEQ, QB], F32, kind="ExternalInput")
    WT = nc.dram_tensor("WT", [EMBED, EMBED], F32R, kind="ExternalInput")
    bias = nc.dram_tensor("bias", [EMBED], F32, kind="ExternalInput")
    ones = nc.dram_tensor("ones", [128, D], F32R, kind="ExternalInput")
    y = nc.dram_tensor("y", [QB, EMBED], F32, kind="ExternalOutput")

    ngrp = KC // exp_group    # exp groups per head

    with tile.TileContext(nc) as tc, \
         nc.allow_low_precision(reason="float32r matmul inputs; fp32 accumulate in PSUM"):
        with tc.tile_pool(name="const", bufs=1) as const, \
             tc.tile_pool(name="kpool", bufs=kpool_bufs) as kpool, \
             tc.tile_pool(name="vpool", bufs=vpool_bufs) as vpool, \
             tc.tile_pool(name="vapool", bufs=vapool_bufs) as vapool, \
             tc.tile_pool(name="probs", bufs=probs_bufs) as probs, \
             tc.tile_pool(name="small", bufs=small_bufs) as small, \
             tc.tile_pool(name="ypool", bufs=ypool_bufs) as ypool, \
             tc.tile_pool(name="scps", bufs=scps_bufs, space="PSUM") as scps, \
             tc.tile_pool(name="accps", bufs=accps_bufs, space="PSUM") as accps:

            # ---- constants ----
            WT_sb = const.tile([128, 8, EMBED], F32R)
            nc.sync.dma_start(out=WT_sb, in_=WT.rearrange("(c p) e -> p c e", p=128))
            mT_sb = const.tile([128, KC, QB], F32)
            nc.sync.dma_start(out=mT_sb, in_=mT.rearrange("(c p) q -> p c q", p=128))
            qT_sb = const.tile([128, 8, QB], F32R)
            nc.sync.dma_start(
                out=qT_sb,
                in_=qT.rearrange("(hp two) d q -> (two d) hp q", two=2))
            bias_ap = bias[:]
            bias_bc = const.tile([128, EMBED], F32)
            nc.sync.dma_start(
                out=bias_bc,
                in_=bass.AP(tensor=bias_ap.tensor, offset=bias_ap.offset,
                            ap=[[0, 128]] + list(bias_ap.ap)),
            )
            ones_sb = const.tile([128, D], F32R)
            nc.sync.dma_start(out=ones_sb, in_=ones[:, :])
            attnT = const.tile([128, 8, QB], F32R)

            v_re = vb.rearrange("(c p) e -> p c e", p=128)

            for hp in range(8):            # head pairs
                kTp = kpool.tile([128, SEQ], F32R)
                nc.sync.dma_start(
                    out=kTp,
                    in_=kT[2 * hp:2 * hp + 2].rearrange("h d s -> (h d) s"))
                vp = vpool.tile([128, KC, 128], F32R)
                nc.sync.dma_start(out=vp,
                                  in_=v_re[:, :, hp * 128:(hp + 1) * 128])
                # assemble [V_h | 1] lhsT tiles on-chip (cheap DVE copies)
                va = vapool.tile([128, KC, 2, D + 1], F32R)
                nc.vector.tensor_copy(va[:, :, 0, 0:D], vp[:, :, 0:D])
                nc.vector.tensor_copy(va[:, :, 1, 0:D], vp[:, :, D:2 * D])
                nc.vector.tensor_copy(va[:, :, :, D], ones_sb[:, 0:KC * 2])

                if interleave:
                    outTs = [accps.tile([D + 1, QB], F32, tag="acc",
                                        name=f"outT{hp}_{hh}")
                             for hh in range(2)]
                    for g in range(8):
                        sc4 = scps.tile([128, 4, QB], F32, tag="sc")
                        for hh in range(2):
                            for j in range(2):
                                c = 2 * g + j
                                nc.tensor.matmul(
                                    sc4[:, 2 * hh + j, :],
                                    kTp[hh * D:hh * D + D, c * 128:(c + 1) * 128],
                                    qT_sb[hh * D:hh * D + D, hp, :],
                                    start=True, stop=True)
                        pe4 = probs.tile([128, 4, QB], F32R, tag="pe")
                        nc.scalar.activation(out=pe4, in_=sc4,
                                             func=mybir.ActivationFunctionType.Exp,
                                             scale=float(1.0 / np.sqrt(D)))
                        for hh in range(2):
                            eng = nc.gpsimd if (gps_mod and (2 * g + hh) % gps_mod == 0) else nc.vector
                            eng.tensor_mul(
                                pe4[:, 2 * hh:2 * hh + 2, :],
                                pe4[:, 2 * hh:2 * hh + 2, :],
                                mT_sb[:, 2 * g:2 * g + 2, :])
                        for hh in range(2):
                            for j in range(2):
                                c = 2 * g + j
                                nc.tensor.matmul(outTs[hh], va[:, c, hh, :],
                                                 pe4[:, 2 * hh + j, :],
                                                 start=(c == 0), stop=(c == KC - 1))
                    for hh in range(2):
                        outT = outTs[hh]
                        rc = small.tile([1, QB], F32R, tag="rc")
                        nc.vector.reciprocal(rc, outT[D:D + 1, :])
                        rb_ps = accps.tile([D, QB], F32, tag="acc")
                        nc.tensor.matmul(rb_ps, ones_sb[0:1, 0:D], rc[0:1, :],
                                         start=True, stop=True)
                        rb_sb = small.tile([D, QB], F32, tag="rb")
                        nc.vector.tensor_copy(rb_sb, rb_ps)
                        nc.vector.tensor_mul(
                            attnT[hh * D:hh * D + D, hp, :],
                            outT[0:D, :], rb_sb)
                    continue
                for hh in range(2):
                    h = 2 * hp + hh
                    outT = accps.tile([D + 1, QB], F32, tag="acc")
                    for g in range(ngrp):
                        sc = scps.tile([128, exp_group, QB], F32, tag="sc")
                        for j in range(exp_group):
                            c = exp_group * g + j
                            nc.tensor.matmul(
                                sc[:, j, :],
                                kTp[hh * D:hh * D + D, c * 128:(c + 1) * 128],
                                qT_sb[hh * D:hh * D + D, hp, :],
                                start=True, stop=True)
                        pe_t = probs.tile([128, exp_group, QB], F32R, tag="pe")
                        nc.scalar.activation(out=pe_t, in_=sc,
                                             func=mybir.ActivationFunctionType.Exp,
                                             scale=float(1.0 / np.sqrt(D)))
                        eng = nc.gpsimd if (gps_mod and g % gps_mod == 0) else nc.vector
                        eng.tensor_mul(
                            pe_t, pe_t,
                            mT_sb[:, exp_group * g:exp_group * (g + 1), :])
                        for j in range(exp_group):
                            c = exp_group * g + j
                            nc.tensor.matmul(outT, va[:, c, hh, :], pe_t[:, j, :],
                                             start=(c == 0), stop=(c == KC - 1))

                    # normalize: recip of denom row, broadcast via PE outer
                    # product (ones64 x recip), evict+scale on DVE
                    rc = small.tile([1, QB], F32R, tag="rc")
                    nc.vector.reciprocal(rc, outT[D:D + 1, :])
                    rb_ps = accps.tile([D, QB], F32, tag="acc")
                    nc.tensor.matmul(rb_ps, ones_sb[0:1, 0:D], rc[0:1, :],
                                     start=True, stop=True)
                    rb_sb = small.tile([D, QB], F32, tag="rb")
                    nc.vector.tensor_copy(rb_sb, rb_ps)
                    nc.vector.tensor_mul(
                        attnT[hh * D:hh * D + D, hp, :],
                        outT[0:D, :], rb_sb)

            # ---- output linear ----
            for qc in range(4):
                y_sb = ypool.tile([128, EMBED], F32)
                for n in range(2):
                    ps = accps.tile([128, 512], F32, tag="acc")
                    for kc in range(8):
                        nc.tensor.matmul(ps,
                                         attnT[:, kc, qc * 128:(qc + 1) * 128],
                                         WT_sb[:, kc, n * 512:(n + 1) * 512],
                                         start=(kc == 0), stop=(kc == 7))
                    nc.vector.tensor_add(y_sb[:, n * 512:(n + 1) * 512], ps,
                                         bias_bc[:, n * 512:(n + 1) * 512])
                nc.sync.dma_start(out=y[qc * 128:(qc + 1) * 128, :], in_=y_sb)

    nc.compile()
    return nc


def _prep_in_maps(q, k, v, padding_mask, W, b):
    q = np.asarray(q, dtype=np.float32)
    k = np.asarray(k, dtype=np.float32)
    v = np.asarray(v, dtype=np.float32)
    m = np.asarray(padding_mask)
    W = np.asarray(W, dtype=np.float32)
    b = np.asarray(b, dtype=np.float32)

    # [bs, seq, embed] -> [bs, heads, d, seq]
    qT = np.ascontiguousarray(q.reshape(BS, SEQ, HEADS, D).transpose(0, 2, 3, 1))
    kT = np.ascontiguousarray(k.reshape(BS, SEQ, HEADS, D).transpose(0, 2, 3, 1))
    # mask [bs, 1, q, k] -> float [bs, k, q]
    mT = np.ascontiguousarray(m[:, 0].transpose(0, 2, 1).astype(np.float32))
    WTc = np.ascontiguousarray(W.T)

    in_maps = []
    for c in range(NC_COUNT):
        bi, qb = c // 4, c % 4
        in_maps.append({
            "qT": np.ascontiguousarray(qT[bi, :, :, qb * QB:(qb + 1) * QB]),
            "kT": kT[bi],
            "vb": v[bi],
            "mT": np.ascontiguousarray(mT[bi, :, qb * QB:(qb + 1) * QB]),
            "WT": WTc,
            "bias": b,
            "ones": np.ones((128, D), dtype=np.float32),
        })
    return in_maps


def _run(in_maps, **kw):
    if "nc" not in _CACHE:
        _CACHE["nc"] = _build_nc()
    return run_bass_kernel_spmd(_CACHE["nc"], in_maps, list(range(NC_COUNT)), **kw)


def kernel(q, k, v, padding_mask, W, b):
    in_maps = _prep_in_maps(q, k, v, padding_mask, W, b)
    res = _run(in_maps)
    out = np.empty((BS, SEQ, EMBED), dtype=np.float32)
    for c in range(NC_COUNT):
        bi, qb = c // 4, c % 4
        out[bi, qb * QB:(qb + 1) * QB] = res.results[c]["y"]
    return out


# revision 14
# speedup vs baseline: 17453.2967x; 1.0452x over previous
"""Multi-head attention + output Linear on 8 Trainium2 NeuronCores.

Problem: bs=2, seq=2048, embed=1024, heads=16, head_dim=64.
  out = Linear(softmax(mask(Q K^T / 8)) V)        (eval-mode dropout)

Sharding: core c in 0..7 handles batch b = c//4 and query block qb = c%4
(512 query rows), computing its exact [512, 1024] output slice - heads stay
together per core so the output Linear needs no cross-core reduction.

Per-core kernel (Tile framework), all matmuls in float32r (1 cyc/row):
  scoresT[k, q] = K_h Q_h^T   (contraction over d=64, partition dim)
  probsT = exp(scoresT / 8) * maskT        (ACT exp fused scale; DVE/GPSIMD mul)
  outT[65, q]  = [V_h | 1]^T probsT        (ones column yields softmax denom)
  attnT = outT[0:64] * (1 / outT[64])      (denom broadcast via PE outer product)
  y = attnT^T W^T + bias                   (accumulate all 16 heads)

Heads are processed in pairs so K / V DMAs move 512B+ descriptors; the
[V | 1] lhsT tiles are assembled on-chip by DVE copies (tiny strided DMAs
were the dominant modeled cost).
"""

import sys
import numpy as np

sys.path.insert(0, "/opt/trn_rl_repo")

import concourse.bass as bass
import concourse.tile as tile
from concourse import bacc, mybir
from concourse.bass_utils import run_bass_kernel_spmd

BS, SEQ, EMBED, HEADS = 2, 2048, 1024, 16
D = EMBED // HEADS            # 64
QB = SEQ // 4                 # 512 query rows per core
NC_COUNT = 8
KC = SEQ // 128               # 16 k chunks
F32 = mybir.dt.float32
F32R = mybir.dt.float32r

_CACHE = {}


def _build_nc(scps_bufs=3, accps_bufs=2, probs_bufs=8, gps_mod=3, kpool_bufs=2,
              vpool_bufs=2, vapool_bufs=2, small_bufs=4, ypool_bufs=2,
              exp_group=2, interleave=False, il_probs_bufs=3):
    nc = bacc.Bacc("TRN2", target_bir_lowering=False, debug=False)

    qT = nc.dram_tensor("qT", [HEADS, D, QB], F32R, kind="ExternalInput")
    kT = nc.dram_tensor("kT", [HEADS, D, SEQ], F32R, kind="ExternalInput")
    vb = nc.dram_tensor("vb", [SEQ, EMBED], F32R, kind="ExternalInput")
    mT = nc.dram_tensor("mT", [SEQ, QB], mybir.dt.bfloat16, kind="ExternalInput")
    WT = nc.dram_tensor("WT", [EMBED, EMBED], F32R, kind="ExternalInput")
    bias = nc.dram_tensor("bias", [EMBED], F32, kind="ExternalInput")
    ones = nc.dram_tensor("ones", [128, D], F32R, kind="ExternalInput")
    y = nc.dram_tensor("y", [QB, EMBED], F32, kind="ExternalOutput")

    ngrp = KC // exp_group    # exp groups per head

    with tile.TileContext(nc) as tc, \
         nc.allow_low_precision(reason="float32r matmul inputs; fp32 accumulate in PSUM"):
        with tc.tile_pool(name="const", bufs=1) as const, \
             tc.tile_pool(name="kpool", bufs=kpool_bufs) as kpool, \
             tc.tile_pool(name="vpool", bufs=vpool_bufs) as vpool, \
             tc.tile_pool(name="vapool", bufs=vapool_bufs) as vapool, \
             tc.tile_pool(name="probs", bufs=probs_bufs) as probs, \
             tc.tile_pool(name="small", bufs=small_bufs) as small, \
             tc.tile_pool(name="ypool", bufs=ypool_bufs) as ypool, \
             tc.tile_pool(name="scps", bufs=scps_bufs, space="PSUM") as scps, \
             tc.tile_pool(name="accps", bufs=accps_bufs, space="PSUM") as accps:

            # ---- constants ----
            WT_sb = const.tile([128, 8, EMBED], F32R)
            nc.sync.dma_start(out=WT_sb, in_=WT.rearrange("(c p) e -> p c e", p=128))
            mT_sb = const.tile([128, KC, QB], mybir.dt.bfloat16)
            nc.sync.dma_start(out=mT_sb, in_=mT.rearrange("(c p) q -> p c q", p=128))
            qT_sb = const.tile([128, 8, QB], F32R)
            nc.sync.dma_start(
                out=qT_sb,
                in_=qT.rearrange("(hp two) d q -> (two d) hp q", two=2))
            bias_ap = bias[:]
            bias_bc = const.tile([128, EMBED], F32)
            nc.sync.dma_start(
                out=bias_bc,
                in_=bass.AP(tensor=bias_ap.tensor, offset=bias_ap.offset,
                            ap=[[0, 128]] + list(bias_ap.ap)),
            )
            ones_sb = const.tile([128, D], F32R)
            nc.sync.dma_start(out=ones_sb, in_=ones[:, :])
            attnT = const.tile([128, 8, QB], F32R)

            v_re = vb.rearrange("(c p) e -> p c e", p=128)

            for hp in range(8):            # head pairs
                kTp = kpool.tile([128, SEQ], F32R)
                nc.sync.dma_start(
                    out=kTp,
                    in_=kT[2 * hp:2 * hp + 2].rearrange("h d s -> (h d) s"))
                vp = vpool.tile([128, KC, 128], F32R)
                nc.sync.dma_start(out=vp,
                                  in_=v_re[:, :, hp * 128:(hp + 1) * 128])
                # assemble [V_h | 1] lhsT tiles on-chip (cheap DVE copies)
                va = vapool.tile([128, KC, 2, D + 1], F32R)
                nc.vector.tensor_copy(va[:, :, 0, 0:D], vp[:, :, 0:D])
                nc.vector.tensor_copy(va[:, :, 1, 0:D], vp[:, :, D:2 * D])
                nc.vector.tensor_copy(va[:, :, :, D], ones_sb[:, 0:KC * 2])

                if interleave:
                    outTs = [accps.tile([D + 1, QB], F32, tag="acc",
                                        name=f"outT{hp}_{hh}")
                             for hh in range(2)]
                    for g in range(8):
                        sc4 = scps.tile([128, 4, QB], F32, tag="sc")
                        for hh in range(2):
                            for j in range(2):
                                c = 2 * g + j
                                nc.tensor.matmul(
                                    sc4[:, 2 * hh + j, :],
                                    kTp[hh * D:hh * D + D, c * 128:(c + 1) * 128],
                                    qT_sb[hh * D:hh * D + D, hp, :],
                                    start=True, stop=True)
                        pe4 = probs.tile([128, 4, QB], F32R, tag="pe")
                        nc.scalar.activation(out=pe4, in_=sc4,
                                             func=mybir.ActivationFunctionType.Exp,
                                             scale=float(1.0 / np.sqrt(D)))
                        for hh in range(2):
                            eng = nc.gpsimd if (gps_mod and (2 * g + hh) % gps_mod == 0) else nc.vector
                            eng.tensor_mul(
                                pe4[:, 2 * hh:2 * hh + 2, :],
                                pe4[:, 2 * hh:2 * hh + 2, :],
                                mT_sb[:, 2 * g:2 * g + 2, :])
                        for hh in range(2):
                            for j in range(2):
                                c = 2 * g + j
                                nc.tensor.matmul(outTs[hh], va[:, c, hh, :],
                                                 pe4[:, 2 * hh + j, :],
                                                 start=(c == 0), stop=(c == KC - 1))
                    for hh in range(2):
                        outT = outTs[hh]
                        rc = small.tile([1, QB], F32R, tag="rc")
                        nc.vector.reciprocal(rc, outT[D:D + 1, :])
                        rb_ps = accps.tile([D, QB], F32, tag="acc")
                        nc.tensor.matmul(rb_ps, ones_sb[0:1, 0:D], rc[0:1, :],
                                         start=True, stop=True)
                        rb_sb = small.tile([D, QB], F32, tag="rb")
                        nc.vector.tensor_copy(rb_sb, rb_ps)
                        nc.vector.tensor_mul(
                            attnT[hh * D:hh * D + D, hp, :],
                            outT[0:D, :], rb_sb)
                    continue
                for hh in range(2):
                    h = 2 * hp + hh
                    outT = accps.tile([D + 1, QB], F32, tag="acc")
                    for g in range(ngrp):
                        sc = scps.tile([128, exp_group, QB], F32, tag="sc")
                        for j in range(exp_group):
                            c = exp_group * g + j
                            nc.tensor.matmul(
                                sc[:, j, :],
                                kTp[hh * D:hh * D + D, c * 128:(c + 1) * 128],
                                qT_sb[hh * D:hh * D + D, hp, :],
                                start=True, stop=True)
                        pe_t = probs.tile([128, exp_group, QB], F32R, tag="pe")
                        nc.scalar.activation(out=pe_t, in_=sc,
                                             func=mybir.ActivationFunctionType.Exp,
                                             scale=float(1.0 / np.sqrt(D)))
                        eng = nc.gpsimd if (gps_mod and g % gps_mod == 0) else nc.vector
                        eng.tensor_mul(
                            pe_t, pe_t,
                            mT_sb[:, exp_group * g:exp_group * (g + 1), :])
                        for j in range(exp_group):
                            c = exp_group * g + j
                            nc.tensor.matmul(outT, va[:, c, hh, :], pe_t[:, j, :],
                                             start=(c == 0), stop=(c == KC - 1))

                    # normalize: recip of denom row, broadcast via PE outer
                    # product (ones64 x recip), evict+scale on DVE
                    rc = small.tile([1, QB], F32R, tag="rc")
                    nc.vector.reciprocal(rc, outT[D:D + 1, :])
                    rb_ps = accps.tile([D, QB], F32, tag="acc")
                    nc.tensor.matmul(rb_ps, ones_sb[0:1, 0:D], rc[0:1, :],
                                     start=True, stop=True)
                    rb_sb = small.tile([D, QB], F32, tag="rb")
                    nc.vector.tensor_copy(rb_sb, rb_ps)
                    nc.vector.tensor_mul(
                        attnT[hh * D:hh * D + D, hp, :],
                        outT[0:D, :], rb_sb)

            # ---- output linear ----
            for qc in range(4):
                y_sb = ypool.tile([128, EMBED], F32)
                for n in range(2):
                    ps = accps.tile([128, 512], F32, tag="acc")
                    for kc in range(8):
                        nc.tensor.matmul(ps,
                                         attnT[:, kc, qc * 128:(qc + 1) * 128],
                                         WT_sb[:, kc, n * 512:(n + 1) * 512],
                                         start=(kc == 0), stop=(kc == 7))
                    nc.vector.tensor_add(y_sb[:, n * 512:(n + 1) * 512], ps,
                                         bias_bc[:, n * 512:(n + 1) * 512])
                nc.sync.dma_start(out=y[qc * 128:(qc + 1) * 128, :], in_=y_sb)

    nc.compile()
    return nc


def _prep_in_maps(q, k, v, padding_mask, W, b):
    q = np.asarray(q, dtype=np.float32)
    k = np.asarray(k, dtype=np.float32)
    v = np.asarray(v, dtype=np.float32)
    m = np.asarray(padding_mask)
    W = np.asarray(W, dtype=np.float32)
    b = np.asarray(b, dtype=np.float32)

    # [bs, seq, embed] -> [bs, heads, d, seq]
    qT = np.ascontiguousarray(q.reshape(BS, SEQ, HEADS, D).transpose(0, 2, 3, 1))
    kT = np.ascontiguousarray(k.reshape(BS, SEQ, HEADS, D).transpose(0, 2, 3, 1))
    # mask [bs, 1, q, k] -> float [bs, k, q]
    import ml_dtypes
    mT = np.ascontiguousarray(m[:, 0].transpose(0, 2, 1).astype(ml_dtypes.bfloat16))
    WTc = np.ascontiguousarray(W.T)

    in_maps = []
    for c in range(NC_COUNT):
        bi, qb = c // 4, c % 4
        in_maps.append({
            "qT": np.ascontiguousarray(qT[bi, :, :, qb * QB:(qb + 1) * QB]),
            "kT": kT[bi],
            "vb": v[bi],
            "mT": np.ascontiguousarray(mT[bi, :, qb * QB:(qb + 1) * QB]),
            "WT": WTc,
            "bias": b,
            "ones": np.ones((128, D), dtype=np.float32),
        })
    return in_maps


def _run(in_maps, **kw):
    if "nc" not in _CACHE:
        _CACHE["nc"] = _build_nc()
    return run_bass_kernel_spmd(_CACHE["nc"], in_maps, list(range(NC_COUNT)), **kw)


def kernel(q, k, v, padding_mask, W, b):
    in_maps = _prep_in_maps(q, k, v, padding_mask, W, b)
    res = _run(in_maps)
    out = np.empty((BS, SEQ, EMBED), dtype=np.float32)
    for c in range(NC_COUNT):
        bi, qb = c // 4, c % 4
        out[bi, qb * QB:(qb + 1) * QB] = res.results[c]["y"]
    return out
